# revision 10
# baseline (speedup 1.0000x reference)
"""CapsuleLayer forward (squash + per-capsule matmul) on 8 Trainium2 cores.

Reference computation (all fp32):
    x  = inputs.reshape(B, 1152, 8)
    pc = squash(x)                              # per-(b,n) over k=8
    u_hat[b,n,j,d] = sum_k W[0,n,j,d,k] * pc[b,n,k]
    out = u_hat[..., None]                      # [B, 1152, 10, 16, 1]

Sharding: capsule dim (n=1152) split 144-per-core across 8 cores; every core
keeps the full batch (B=512).  Zero cross-device communication.

Per-core kernel (fp16 data paths; PSUM accumulates fp32):
  - squash scale simplified algebraically: sq/((1+sq)*sqrt(sq+eps))
    == sqrt(sq)/(1+sq) (eps negligible, NaN-safe at sq=0)
  - the scale chain runs in 9 per-16-cap SEGMENTS per chunk: chunk-0
    segments 0-1 on DVE (latency-critical path to the first matmul),
    everything else on the otherwise-idle GpSimd (tree-sum over k=8, no
    free-axis reduce there) with only sqrt on ACT and the reciprocal on
    DVE -- keeps the DVE/ACT evacuation lanes under the store rate
  - W host-packed as flat K=128 16-cap block-diagonal [128, 2560] tiles
    (16x zero-padding, 5.9MB).  K=128 is mandatory for speed: K=64
    matmuls never reach the PE's 2.4GHz boost clock (measured: 0/144 fast
    vs ~50%+ for K=128) and run a permanent 2x slower at 1.2GHz, which
    costs far more than the extra 3MB of one-time W traffic
  - x chunk 0 + W groups 0-4 on the sync ring, x chunks 1-3 + W groups
    5-8 on the scalar ring: W streams in behind the x prefetches on BOTH
    rings and is fully resident by ~19us, just ahead of the PE's
    consumption, without delaying chunk-0 compute
  - pc transposed via PE ([128,128] identity matmul), pipelined one group
    ahead
  - per group 5 512-col matmuls sharing one stationary pcT; PSUM->SBUF
    evacuation pa->DVE, pb->ACT, pcs alternating; output stored per group
    in [128, 2560] fp16 tiles alternating the sync/scalar HWDGE rings so
    SDMA engines round-robin two queue rows and hide per-DMA
    completion-receipt bubbles
"""

from contextlib import ExitStack

import numpy as np

import concourse.bacc as bacc
import concourse.bass as bass  # noqa: F401  (AP helpers)
import concourse.mybir as mybir
import concourse.tile as tile
from concourse.bass_utils import run_bass_kernel_spmd
from concourse.masks import make_identity

N_CORES = 8
B = 512
N_CAPS = 1152
K = 8
JD = 160  # 10*16
CAPS_PER_CORE = N_CAPS // N_CORES  # 144
GROUP_CAPS = 16  # caps per output group -> 2560 cols
N_GROUPS = CAPS_PER_CORE // GROUP_CAPS  # 9
GROUP_COLS = GROUP_CAPS * JD  # 2560
P = 128
B_CHUNKS = B // P  # 4

F32 = mybir.dt.float32
F16 = mybir.dt.float16
OUT_DT = mybir.dt.float16


def build_program():
    nc = bacc.Bacc("TRN2", debug=False, num_devices=N_CORES)
    x = nc.dram_tensor("x", [B, CAPS_PER_CORE * K], F16, kind="ExternalInput").ap()
    wb = nc.dram_tensor(
        "wb", [N_GROUPS * P, GROUP_COLS], F16, kind="ExternalInput"
    ).ap()
    out = nc.dram_tensor(
        "out", [B, CAPS_PER_CORE * JD], OUT_DT, kind="ExternalOutput"
    ).ap()

    with tile.TileContext(nc) as tc, ExitStack() as ctx:
        consts = ctx.enter_context(tc.tile_pool(name="consts", bufs=1))
        wblk_pool = ctx.enter_context(tc.tile_pool(name="wblk", bufs=1))
        xpool = ctx.enter_context(tc.tile_pool(name="xpool", bufs=4))
        x2pool = ctx.enter_context(tc.tile_pool(name="x2pool", bufs=2))
        x2poolg = ctx.enter_context(tc.tile_pool(name="x2poolg", bufs=2))
        pcpool = ctx.enter_context(tc.tile_pool(name="pcpool", bufs=2))
        stats = ctx.enter_context(tc.tile_pool(name="stats", bufs=2))
        statsg = ctx.enter_context(tc.tile_pool(name="statsg", bufs=2))
        pct_pool = ctx.enter_context(tc.tile_pool(name="pct", bufs=3))
        ost_pool = ctx.enter_context(tc.tile_pool(name="ost", bufs=6))
        # PSUM: 3x 2-bank matmul slots + 2x 1-bank transpose slots = 8 banks.
        psum = ctx.enter_context(tc.tile_pool(name="psum", bufs=3, space="PSUM"))
        psum_t = ctx.enter_context(tc.tile_pool(name="psum_t", bufs=2, space="PSUM"))

        # x chunk 0 at the head of the sync ring: the scalar ring's first
        # slots go to ACT table loads, so sync gets x0 out ~1us earlier.
        xts = []
        xt0 = xpool.tile([P, CAPS_PER_CORE, K], F16, tag="xt0")
        nc.sync.dma_start(
            out=xt0, in_=x[0:P, :].rearrange("b (c k) -> b c k", k=K)
        )
        xts.append(xt0)
        for bi in range(1, B_CHUNKS):
            xt = xpool.tile([P, CAPS_PER_CORE, K], F16, tag=f"xt{bi}")
            nc.scalar.dma_start(
                out=xt,
                in_=x[bi * P : (bi + 1) * P, :].rearrange("b (c k) -> b c k", k=K),
            )
            xts.append(xt)
        wblk = []
        for g in range(N_GROUPS):
            wt = wblk_pool.tile(
                [P, GROUP_COLS], F16, tag=f"wblk{g}", name=f"wblk{g}"
            )
            eng = nc.sync if g < 5 else nc.scalar
            eng.dma_start(out=wt, in_=wb[g * P : (g + 1) * P, :])
            wblk.append(wt)

        identity = consts.tile([P, P], F16)
        make_identity(nc, identity)

        def emit_seg(xt, pc, c0, c1, eng):
            # scale[b,c] = sqrt(sq)/(1+sq); pc = x*scale over caps [c0,c1).
            # `eng` (DVE or GpSimd) does the elementwise work; sqrt is ACT,
            # reciprocal DVE.  Batched ranges amortize per-op overhead.
            ncap = c1 - c0
            xs = xt[:, c0:c1, :]
            gp = eng is nc.gpsimd
            xp = x2poolg if gp else x2pool
            st = statsg if gp else stats
            x2 = xp.tile([P, ncap, K], F16, tag=f"x2_{ncap}", name="x2")
            eng.tensor_mul(x2, xs, xs)
            sq = st.tile([P, ncap], F16, tag=f"sq_{ncap}", name="sq")
            if gp:
                s4 = st.tile([P, ncap, 4], F16, tag=f"s4_{ncap}", name="s4")
                eng.tensor_add(s4, x2[:, :, 0:4], x2[:, :, 4:8])
                s2 = st.tile([P, ncap, 2], F16, tag=f"s2_{ncap}", name="s2")
                eng.tensor_add(s2, s4[:, :, 0:2], s4[:, :, 2:4])
                eng.tensor_add(sq.unsqueeze(2), s2[:, :, 0:1], s2[:, :, 1:2])
            else:
                eng.reduce_sum(out=sq, in_=x2, axis=mybir.AxisListType.X)
            sn = st.tile([P, ncap], F16, tag=f"sn_{ncap}", name="sn")
            nc.scalar.activation(
                out=sn, in_=sq, func=mybir.ActivationFunctionType.Sqrt
            )
            t1 = st.tile([P, ncap], F16, tag=f"t1_{ncap}", name="t1")
            eng.tensor_scalar_add(t1, sq, 1.0)
            rden = st.tile([P, ncap], F16, tag=f"rd_{ncap}", name="rd")
            nc.vector.reciprocal(rden, t1)
            scale = st.tile([P, ncap], F16, tag=f"sc_{ncap}", name="sc")
            eng.tensor_mul(scale, sn, rden)
            eng.tensor_mul(
                pc[:, c0:c1, :],
                xs,
                scale.unsqueeze(2).broadcast_to([P, ncap, K]),
            )

        def issue_transpose(pc_flat, g):
            # Pipelined one group ahead so the PE never waits on the
            # PSUM->SBUF pcT copy.
            pst = psum_t.tile([P, P], F16, tag="pt")
            nc.tensor.transpose(pst, pc_flat[:, g * P : (g + 1) * P], identity)
            pcT = pct_pool.tile([P, P], F16, tag="pcT", name="pcT")
            if g % 2 == 0:
                nc.scalar.copy(pcT, pst)
            else:
                nc.vector.tensor_copy(pcT, pst)
            return pcT

        with nc.allow_low_precision("fp16 squash: tolerance is 2e-2"):
            pc0 = pcpool.tile([P, CAPS_PER_CORE, K], F16, tag="pc")
            # Chunk-0: caps 0-31 on DVE (shortest path to the first
            # matmul), the rest in two batched GpSimd emissions that stay
            # just ahead of the PE while DVE keeps its evac lanes free.
            emit_seg(xts[0], pc0, 0, 2 * GROUP_CAPS, nc.vector)
            emit_seg(xts[0], pc0, 2 * GROUP_CAPS, 5 * GROUP_CAPS, nc.gpsimd)
            emit_seg(xts[0], pc0, 5 * GROUP_CAPS, CAPS_PER_CORE, nc.gpsimd)

            pc_cur = pc0.rearrange("p c k -> p (c k)")
            pc_next = None
            pcn = None
            for bi in range(B_CHUNKS):
                pcT_next = issue_transpose(pc_cur, 0)
                for g in range(N_GROUPS):
                    pcT = pcT_next
                    if g + 1 < N_GROUPS:
                        pcT_next = issue_transpose(pc_cur, g + 1)

                    if bi + 1 < B_CHUNKS and g in (1, 4, 7):
                        # Drip the next chunk's scale chain through GpSimd
                        # in three 3-group batches.
                        if g == 1:
                            pcn = pcpool.tile(
                                [P, CAPS_PER_CORE, K], F16, tag="pc"
                            )
                            pc_next = pcn.rearrange("p c k -> p (c k)")
                        s0 = (g - 1) // 3 * 3 * GROUP_CAPS
                        emit_seg(
                            xts[bi + 1], pcn, s0, s0 + 3 * GROUP_CAPS, nc.gpsimd
                        )

                    pa = psum.tile([P, 1024], F32, tag="pm")
                    pb = psum.tile([P, 1024], F32, tag="pm")
                    pcs = psum.tile([P, 512], F32, tag="pm")
                    for s in range(2):
                        nc.tensor.matmul(
                            pa[:, s * 512 : (s + 1) * 512],
                            lhsT=pcT,
                            rhs=wblk[g][:, s * 512 : (s + 1) * 512],
                            start=True,
                            stop=True,
                        )
                    for s in range(2):
                        nc.tensor.matmul(
                            pb[:, s * 512 : (s + 1) * 512],
                            lhsT=pcT,
                            rhs=wblk[g][:, (2 + s) * 512 : (3 + s) * 512],
                            start=True,
                            stop=True,
                        )
                    nc.tensor.matmul(
                        pcs, lhsT=pcT, rhs=wblk[g][:, 4 * 512 : 5 * 512],
                        start=True, stop=True,
                    )

                    ost = ost_pool.tile([P, GROUP_COLS], OUT_DT)
                    # pa evacuates first on DVE (the next group's first
                    # matmul reuses its bank pair); pb on ACT, pcs alternates.
                    nc.vector.tensor_copy(ost[:, 0:1024], pa)
                    nc.scalar.copy(ost[:, 1024:2048], pb)
                    if g % 2 == 0:
                        nc.vector.tensor_copy(ost[:, 2048:2560], pcs)
                    else:
                        nc.scalar.copy(ost[:, 2048:2560], pcs)
                    # Alternate the two HWDGE rings per store.
                    st_eng = nc.sync if (bi * N_GROUPS + g) % 2 == 0 else nc.scalar
                    st_eng.dma_start(
                        out=out[
                            bi * P : (bi + 1) * P,
                            g * GROUP_COLS : (g + 1) * GROUP_COLS,
                        ],
                        in_=ost,
                    )
                pc_cur = pc_next
    nc.compile()
    return nc


_PROGRAM = None


def _get_program():
    global _PROGRAM
    if _PROGRAM is None:
        _PROGRAM = build_program()
    return _PROGRAM


def shard_inputs(inputs: np.ndarray, W: np.ndarray) -> list[dict[str, np.ndarray]]:
    # Flat K=128 16-cap block-diagonal W per core: per group one
    # [128, 2560] tile; rows (c,k) = c*8+k with W[n=c] at cols
    # c*160..(c+1)*160, zeros elsewhere.
    w0 = np.asarray(W[0], dtype=np.float32).reshape(N_CAPS, JD, K)
    x16 = np.asarray(inputs, dtype=np.float16)
    in_maps = []
    for i in range(N_CORES):
        c0 = i * CAPS_PER_CORE
        wcore = w0[c0 : c0 + CAPS_PER_CORE]  # [144, 160, 8]
        wbd = np.zeros(
            (N_GROUPS, GROUP_CAPS, K, GROUP_CAPS, JD), dtype=np.float16
        )
        for c in range(GROUP_CAPS):
            # wbd[g, c, k, c, jd] = W[g*16+c, jd, k]
            wbd[:, c, :, c, :] = (
                wcore.reshape(N_GROUPS, GROUP_CAPS, JD, K)[:, c]
                .transpose(0, 2, 1)
            )
        in_maps.append(
            {
                "x": np.ascontiguousarray(
                    x16[:, c0 * K : (c0 + CAPS_PER_CORE) * K]
                ),
                "wb": wbd.reshape(N_GROUPS * P, GROUP_COLS),
            }
        )
    return in_maps


def unshard_output(results: list[dict[str, np.ndarray]]) -> np.ndarray:
    full = np.empty((B, N_CAPS, JD), dtype=np.float32)
    for i in range(N_CORES):
        c0 = i * CAPS_PER_CORE
        full[:, c0 : c0 + CAPS_PER_CORE, :] = results[i]["out"].reshape(
            B, CAPS_PER_CORE, JD
        ).astype(np.float32)
    return full.reshape(B, N_CAPS, 10, 16, 1)


def kernel(inputs: np.ndarray, W: np.ndarray) -> np.ndarray:
    nc = _get_program()
    in_maps = shard_inputs(np.asarray(inputs), np.asarray(W))
    res = run_bass_kernel_spmd(nc, in_maps, core_ids=list(range(N_CORES)))
    return unshard_output(res.results)
